# revision 9
# baseline (speedup 1.0000x reference)
"""v3 known-good: dual-ring streaming, all compute on ScalarE."""

import numpy as np

import concourse.bacc as bacc
import concourse.mybir as mybir
from concourse.tile import TileContext
from concourse.bass_utils import run_bass_kernel_spmd

N = 8192
NCORES = 8
ROWS = N // NCORES
P = 128
TILES = ROWS // P

_DT = mybir.dt.float32

TAPER = [4096, 2048, 1536, 512]
assert sum(TAPER) == N

CHUNKS = []
for t in range(6):
    CHUNKS.append((t, 0, N, t % 2))
_off = 0
for w in TAPER:
    CHUNKS.append((6, _off, w, 0))
    CHUNKS.append((7, _off, w, 1))
    _off += w
NCH = len(CHUNKS)  # 14


def build_nc():
    nc = bacc.Bacc("TRN2", target_bir_lowering=False)

    a_shard = nc.dram_tensor("a_shard", [ROWS, N], _DT, kind="ExternalInput")
    out = nc.dram_tensor("out", [P, NCH], _DT, kind="ExternalOutput")

    a_tiles = a_shard.rearrange("(t p) n -> t p n", p=P)

    with TileContext(nc) as tc:
        with (
            tc.tile_pool(name="big", bufs=4) as big,
            tc.tile_pool(name="t4096", bufs=2) as t4096,
            tc.tile_pool(name="t2048", bufs=2) as t2048,
            tc.tile_pool(name="t1536", bufs=2) as t1536,
            tc.tile_pool(name="t512", bufs=2) as t512,
            tc.tile_pool(name="small", bufs=1) as small,
        ):
            pools = {N: (big, "big"), 4096: (t4096, "t4"), 2048: (t2048, "t2"),
                     1536: (t1536, "t1"), 512: (t512, "t5")}
            racc = small.tile([P, NCH], _DT, tag="racc")
            dummy = small.tile([P, 1], _DT, tag="dummy")

            for i, (t, c0, w, ring) in enumerate(CHUNKS):
                pool, tag = pools[w]
                at = pool.tile([P, w], _DT, tag=tag, name=f"at_{tag}")
                eng = nc.sync if ring == 0 else nc.scalar
                eng.dma_start(out=at[:], in_=a_tiles[t][:, c0 : c0 + w])
                nc.scalar.activation(
                    out=dummy.broadcast_to(at.shape),
                    in_=at[:],
                    func=mybir.ActivationFunctionType.Square,
                    accum_out=racc[:, i : i + 1],
                )

            nc.sync.dma_start(out=out[:], in_=racc[:])

    nc.compile()
    return nc


_nc_cache = {}


def _get_nc():
    if "nc" not in _nc_cache:
        _nc_cache["nc"] = build_nc()
    return _nc_cache["nc"]


def _run(inputs, trace=False):
    X = np.ascontiguousarray(np.asarray(inputs["X"], dtype=np.float32))
    A = np.ascontiguousarray(np.asarray(inputs["A"], dtype=np.float32))

    nc = _get_nc()
    in_maps = [
        {"a_shard": A[c * ROWS : (c + 1) * ROWS]} for c in range(NCORES)
    ]
    res = run_bass_kernel_spmd(
        nc, in_maps, core_ids=list(range(NCORES)), trace=trace
    )

    X64 = X.astype(np.float64)
    total = 0.0
    for c in range(NCORES):
        r = res.results[c]["out"].astype(np.float64)
        rt = np.empty((P, TILES), dtype=np.float64)
        rt[:, :6] = r[:, :6]
        rt[:, 6] = r[:, [6, 8, 10, 12]].sum(axis=1)
        rt[:, 7] = r[:, [7, 9, 11, 13]].sum(axis=1)
        xc = X64[c * ROWS : (c + 1) * ROWS].reshape(TILES, P).T
        total += (xc * xc * rt).sum()

    d64 = np.asarray(A.diagonal(), dtype=np.float64)
    total += -2.0 * float(X64 @ d64) + float(N)
    return np.float32(total), res


def kernel(**inputs):
    out, _ = _run(inputs, trace=False)
    return out


# revision 10
# speedup vs baseline: 1.0691x; 1.0691x over previous
"""v4-noDVE bisect: dual-ring 2MiB chunks, upfront issue, all-ACT compute."""

import numpy as np

import concourse.bacc as bacc
import concourse.mybir as mybir
from concourse.tile import TileContext
from concourse.bass_utils import run_bass_kernel_spmd

N = 8192
NCORES = 8
ROWS = N // NCORES
P = 128
TILES = ROWS // P

_DT = mybir.dt.float32

BIG = 4096
TAPER = [2048, 1024, 512, 512]
assert sum(TAPER) == BIG


def _ring_chunks(tiles):
    ch = []
    for t in tiles[:-1]:
        ch.append((t, 0, BIG))
        ch.append((t, BIG, BIG))
    t = tiles[-1]
    ch.append((t, 0, BIG))
    off = BIG
    for w in TAPER:
        ch.append((t, off, w))
        off += w
    return ch

CHUNKS_A = _ring_chunks([0, 2, 4, 6])
CHUNKS_B = _ring_chunks([1, 3, 5, 7])
NCHA = len(CHUNKS_A)  # 11
NCH = NCHA + len(CHUNKS_B)  # 22


def build_nc():
    nc = bacc.Bacc("TRN2", target_bir_lowering=False)

    a_shard = nc.dram_tensor("a_shard", [ROWS, N], _DT, kind="ExternalInput")
    out = nc.dram_tensor("out", [P, NCH], _DT, kind="ExternalOutput")

    a_tiles = a_shard.rearrange("(t p) n -> t p n", p=P)

    with TileContext(nc) as tc:
        with (
            tc.tile_pool(name="bigA", bufs=3) as bigA,
            tc.tile_pool(name="bigB", bufs=3) as bigB,
            tc.tile_pool(name="t2048", bufs=2) as t2048,
            tc.tile_pool(name="t1024", bufs=2) as t1024,
            tc.tile_pool(name="t512", bufs=4) as t512,
            tc.tile_pool(name="small", bufs=1) as small,
        ):
            tpools = {2048: t2048, 1024: t1024, 512: t512}
            racc = small.tile([P, NCH], _DT, tag="racc")
            dummy = small.tile([P, 1], _DT, tag="dummy")

            def alloc(ring_pool, ring_tag, w):
                if w == BIG:
                    return ring_pool.tile(
                        [P, w], _DT, tag=ring_tag, name=f"at_{ring_tag}"
                    )
                return tpools[w].tile(
                    [P, w], _DT, tag=f"{ring_tag}{w}", name=f"at_{ring_tag}{w}"
                )

            atA = []
            for t, c0, w in CHUNKS_A:
                at = alloc(bigA, "a", w)
                nc.sync.dma_start(out=at[:], in_=a_tiles[t][:, c0 : c0 + w])
                atA.append(at)

            atB = []

            def issue_b(k):
                t, c0, w = CHUNKS_B[k]
                at = alloc(bigB, "b", w)
                nc.scalar.dma_start(out=at[:], in_=a_tiles[t][:, c0 : c0 + w])
                atB.append(at)

            def act(tile_ap, col):
                nc.scalar.activation(
                    out=dummy.broadcast_to(tile_ap.shape),
                    in_=tile_ap[:],
                    func=mybir.ActivationFunctionType.Square,
                    accum_out=racc[:, col : col + 1],
                )

            for k in range(3):
                issue_b(k)
            # Alternate A/B consumption on ACT, keeping 3 ring-B issues
            # in hand ahead of the activations that would block them.
            for k in range(NCHA):
                act(atA[k], k)
                if k + 3 < NCHA:
                    issue_b(k + 3)
                act(atB[k], NCHA + k)

            nc.sync.dma_start(out=out[:], in_=racc[:])

    nc.compile()
    return nc


_nc_cache = {}


def _get_nc():
    if "nc" not in _nc_cache:
        _nc_cache["nc"] = build_nc()
    return _nc_cache["nc"]


_COL_TILE = np.array([t for t, _, _ in CHUNKS_A] + [t for t, _, _ in CHUNKS_B])


def _run(inputs, trace=False):
    X = np.ascontiguousarray(np.asarray(inputs["X"], dtype=np.float32))
    A = np.ascontiguousarray(np.asarray(inputs["A"], dtype=np.float32))

    nc = _get_nc()
    in_maps = [
        {"a_shard": A[c * ROWS : (c + 1) * ROWS]} for c in range(NCORES)
    ]
    res = run_bass_kernel_spmd(
        nc, in_maps, core_ids=list(range(NCORES)), trace=trace
    )

    X64 = X.astype(np.float64)
    total = 0.0
    for c in range(NCORES):
        r = res.results[c]["out"].astype(np.float64)
        rt = np.zeros((P, TILES), dtype=np.float64)
        for col in range(NCH):
            rt[:, _COL_TILE[col]] += r[:, col]
        xc = X64[c * ROWS : (c + 1) * ROWS].reshape(TILES, P).T
        total += (xc * xc * rt).sum()

    d64 = np.asarray(A.diagonal(), dtype=np.float64)
    total += -2.0 * float(X64 @ d64) + float(N)
    return np.float32(total), res


def kernel(**inputs):
    out, _ = _run(inputs, trace=False)
    return out


# revision 12
# speedup vs baseline: 1.1501x; 1.0758x over previous
"""v4-noDVE bisect: dual-ring 2MiB chunks, upfront issue, all-ACT compute."""

import numpy as np

import concourse.bacc as bacc
import concourse.mybir as mybir
from concourse.tile import TileContext
from concourse.bass_utils import run_bass_kernel_spmd

N = 8192
NCORES = 8
ROWS = N // NCORES
P = 128
TILES = ROWS // P

_DT = mybir.dt.float32

BIG = 4096
TAPER = [2048, 1024, 512, 512]
assert sum(TAPER) == BIG


def _ring_chunks(tiles):
    ch = []
    for t in tiles[:-1]:
        ch.append((t, 0, BIG))
        ch.append((t, BIG, BIG))
    t = tiles[-1]
    ch.append((t, 0, BIG))
    off = BIG
    for w in TAPER:
        ch.append((t, off, w))
        off += w
    return ch

CHUNKS_A = _ring_chunks([0, 2, 4, 6])
CHUNKS_B = _ring_chunks([1, 3, 5, 7])
NCHA = len(CHUNKS_A)  # 11
NCH = NCHA + len(CHUNKS_B)  # 22


def build_nc():
    nc = bacc.Bacc("TRN2", target_bir_lowering=False)

    a_shard = nc.dram_tensor("a_shard", [ROWS, N], _DT, kind="ExternalInput")
    out = nc.dram_tensor("out", [P, NCH], _DT, kind="ExternalOutput")

    a_tiles = a_shard.rearrange("(t p) n -> t p n", p=P)

    with TileContext(nc) as tc:
        with (
            tc.tile_pool(name="bigA", bufs=3) as bigA,
            tc.tile_pool(name="bigB", bufs=3) as bigB,
            tc.tile_pool(name="t2048", bufs=2) as t2048,
            tc.tile_pool(name="t1024", bufs=2) as t1024,
            tc.tile_pool(name="t512", bufs=4) as t512,
            tc.tile_pool(name="small", bufs=1) as small,
        ):
            tpools = {2048: t2048, 1024: t1024, 512: t512}
            # Separate accumulators per engine so Tile never serializes
            # ScalarE against VectorE through a shared tile.
            racc_a = small.tile([P, NCHA], _DT, tag="racc_a")
            racc_v = small.tile([P, NCH - NCHA], _DT, tag="racc_v")
            dummy = small.tile([P, 1], _DT, tag="dummy")
            # Scratch for VectorE's squared products. (tensor_mul +
            # reduce_sum, NOT tensor_tensor_reduce: TTR's accumulator-
            # read lowers to a raw InstISA op that crashes the HW path.)
            vout = small.tile([P, BIG], _DT, tag="vout")

            def alloc(ring_pool, ring_tag, w):
                if w == BIG:
                    return ring_pool.tile(
                        [P, w], _DT, tag=ring_tag, name=f"at_{ring_tag}"
                    )
                return tpools[w].tile(
                    [P, w], _DT, tag=f"{ring_tag}{w}", name=f"at_{ring_tag}{w}"
                )

            atA = []
            for t, c0, w in CHUNKS_A:
                at = alloc(bigA, "a", w)
                nc.sync.dma_start(out=at[:], in_=a_tiles[t][:, c0 : c0 + w])
                atA.append(at)

            atB = []

            def issue_b(k):
                t, c0, w = CHUNKS_B[k]
                at = alloc(bigB, "b", w)
                nc.scalar.dma_start(out=at[:], in_=a_tiles[t][:, c0 : c0 + w])
                atB.append(at)

            def act(tile_ap, col):
                nc.scalar.activation(
                    out=dummy.broadcast_to(tile_ap.shape),
                    in_=tile_ap[:],
                    func=mybir.ActivationFunctionType.Square,
                    accum_out=racc_a[:, col : col + 1],
                )

            def dve(k):
                at = atB[k]
                w = at.shape[1]
                nc.vector.tensor_mul(out=vout[:, :w], in0=at[:], in1=at[:])
                nc.vector.reduce_sum(
                    racc_v[:, k : k + 1], vout[:, :w], axis=mybir.AxisListType.X
                )

            for k in range(3):
                issue_b(k)
            # ScalarE consumes ring A only; VectorE consumes ring B only.
            # Ring-B DMA triggers ride the ACT stream (it runs well ahead
            # of ring A's ~215 GB/s delivery), 3 issues in hand.
            for k in range(NCHA):
                act(atA[k], k)
                if k + 3 < NCHA:
                    issue_b(k + 3)
                dve(k)

            nc.sync.dma_start(out=out[:, :NCHA], in_=racc_a[:])
            nc.scalar.dma_start(out=out[:, NCHA:], in_=racc_v[:])

    nc.compile()
    return nc


_nc_cache = {}


def _get_nc():
    if "nc" not in _nc_cache:
        _nc_cache["nc"] = build_nc()
    return _nc_cache["nc"]


_COL_TILE = np.array([t for t, _, _ in CHUNKS_A] + [t for t, _, _ in CHUNKS_B])


def _run(inputs, trace=False):
    X = np.ascontiguousarray(np.asarray(inputs["X"], dtype=np.float32))
    A = np.ascontiguousarray(np.asarray(inputs["A"], dtype=np.float32))

    nc = _get_nc()
    in_maps = [
        {"a_shard": A[c * ROWS : (c + 1) * ROWS]} for c in range(NCORES)
    ]
    res = run_bass_kernel_spmd(
        nc, in_maps, core_ids=list(range(NCORES)), trace=trace
    )

    X64 = X.astype(np.float64)
    total = 0.0
    for c in range(NCORES):
        r = res.results[c]["out"].astype(np.float64)
        rt = np.zeros((P, TILES), dtype=np.float64)
        for col in range(NCH):
            rt[:, _COL_TILE[col]] += r[:, col]
        xc = X64[c * ROWS : (c + 1) * ROWS].reshape(TILES, P).T
        total += (xc * xc * rt).sum()

    d64 = np.asarray(A.diagonal(), dtype=np.float64)
    total += -2.0 * float(X64 @ d64) + float(N)
    return np.float32(total), res


def kernel(**inputs):
    out, _ = _run(inputs, trace=False)
    return out


# revision 15
# speedup vs baseline: 1.3638x; 1.1858x over previous
"""v4-noDVE bisect: dual-ring 2MiB chunks, upfront issue, all-ACT compute."""

import numpy as np

import concourse.bacc as bacc
import concourse.mybir as mybir
from concourse.tile import TileContext
from concourse.bass_utils import run_bass_kernel_spmd

N = 8192
NCORES = 8
ROWS = N // NCORES
P = 128
TILES = ROWS // P

_DT = mybir.dt.float32

BIG = 4096
TAPER = [2048, 1024, 512, 512]
assert sum(TAPER) == BIG


def _ring_chunks(tiles):
    ch = []
    for t in tiles[:-1]:
        ch.append((t, 0, BIG))
        ch.append((t, BIG, BIG))
    t = tiles[-1]
    ch.append((t, 0, BIG))
    off = BIG
    for w in TAPER:
        ch.append((t, off, w))
        off += w
    return ch

CHUNKS_A = _ring_chunks([0, 2, 4, 6])
CHUNKS_B = _ring_chunks([1, 3, 5, 7])
NCHA = len(CHUNKS_A)  # 11
NCH = NCHA + len(CHUNKS_B)  # 22


def build_nc():
    nc = bacc.Bacc("TRN2", target_bir_lowering=False)

    a_shard = nc.dram_tensor("a_shard", [ROWS, N], _DT, kind="ExternalInput")
    out = nc.dram_tensor("out", [P, NCH], _DT, kind="ExternalOutput")

    a_tiles = a_shard.rearrange("(t p) n -> t p n", p=P)

    with TileContext(nc) as tc:
        with (
            tc.tile_pool(name="bigA", bufs=3) as bigA,
            tc.tile_pool(name="bigB", bufs=4) as bigB,
            tc.tile_pool(name="t2048", bufs=2) as t2048,
            tc.tile_pool(name="t1024", bufs=2) as t1024,
            tc.tile_pool(name="t512", bufs=4) as t512,
            tc.tile_pool(name="small", bufs=1) as small,
        ):
            tpools = {2048: t2048, 1024: t1024, 512: t512}
            # Separate accumulators per engine so Tile never serializes
            # ScalarE against VectorE through a shared tile.
            racc_a = small.tile([P, NCHA], _DT, tag="racc_a")
            racc_v = small.tile([P, NCH - NCHA], _DT, tag="racc_v")
            dummy = small.tile([P, 1], _DT, tag="dummy")
            # Scratch for VectorE's squared products. (tensor_mul +
            # reduce_sum, NOT tensor_tensor_reduce: TTR's accumulator-
            # read lowers to a raw InstISA op that crashes the HW path.)
            vout = small.tile([P, BIG], _DT, tag="vout")

            def alloc(ring_pool, ring_tag, w):
                if w == BIG:
                    return ring_pool.tile(
                        [P, w], _DT, tag=ring_tag, name=f"at_{ring_tag}"
                    )
                return tpools[w].tile(
                    [P, w], _DT, tag=f"{ring_tag}{w}", name=f"at_{ring_tag}{w}"
                )

            atA = []
            for t, c0, w in CHUNKS_A:
                at = alloc(bigA, "a", w)
                nc.sync.dma_start(out=at[:], in_=a_tiles[t][:, c0 : c0 + w])
                atA.append(at)

            atB = []

            def issue_b(k):
                t, c0, w = CHUNKS_B[k]
                at = alloc(bigB, "b", w)
                nc.scalar.dma_start(out=at[:], in_=a_tiles[t][:, c0 : c0 + w])
                atB.append(at)

            def act(tile_ap, col):
                nc.scalar.activation(
                    out=dummy.broadcast_to(tile_ap.shape),
                    in_=tile_ap[:],
                    func=mybir.ActivationFunctionType.Square,
                    accum_out=racc_a[:, col : col + 1],
                )

            def dve(k):
                # Single-pass square+reduce on VectorE: (at * 1.0) * at
                # with fused accumulator. DVE fp32 measures ~118 G
                # elem/s per pass, so the two-pass mul+reduce variant
                # made VectorE the critical path.
                at = atB[k]
                w = at.shape[1]
                nc.vector.scalar_tensor_tensor(
                    out=vout[:, :w],
                    in0=at[:],
                    scalar=1.0,
                    in1=at[:],
                    op0=mybir.AluOpType.mult,
                    op1=mybir.AluOpType.mult,
                    accum_out=racc_v[:, k : k + 1],
                )

            for k in range(3):
                issue_b(k)
            # ScalarE consumes ring A only; VectorE consumes ring B only.
            # Ring-B DMA triggers ride the ACT stream (it runs well ahead
            # of ring A's ~215 GB/s delivery), 3 issues in hand.
            for k in range(NCHA):
                act(atA[k], k)
                if k + 3 < NCHA:
                    issue_b(k + 3)
                dve(k)

            nc.sync.dma_start(out=out[:, :NCHA], in_=racc_a[:])
            nc.scalar.dma_start(out=out[:, NCHA:], in_=racc_v[:])

    nc.compile()
    return nc


_nc_cache = {}


def _get_nc():
    if "nc" not in _nc_cache:
        _nc_cache["nc"] = build_nc()
    return _nc_cache["nc"]


_COL_TILE = np.array([t for t, _, _ in CHUNKS_A] + [t for t, _, _ in CHUNKS_B])


def _run(inputs, trace=False):
    X = np.ascontiguousarray(np.asarray(inputs["X"], dtype=np.float32))
    A = np.ascontiguousarray(np.asarray(inputs["A"], dtype=np.float32))

    nc = _get_nc()
    in_maps = [
        {"a_shard": A[c * ROWS : (c + 1) * ROWS]} for c in range(NCORES)
    ]
    res = run_bass_kernel_spmd(
        nc, in_maps, core_ids=list(range(NCORES)), trace=trace
    )

    X64 = X.astype(np.float64)
    total = 0.0
    for c in range(NCORES):
        r = res.results[c]["out"].astype(np.float64)
        rt = np.zeros((P, TILES), dtype=np.float64)
        for col in range(NCH):
            rt[:, _COL_TILE[col]] += r[:, col]
        xc = X64[c * ROWS : (c + 1) * ROWS].reshape(TILES, P).T
        total += (xc * xc * rt).sum()

    d64 = np.asarray(A.diagonal(), dtype=np.float64)
    total += -2.0 * float(X64 @ d64) + float(N)
    return np.float32(total), res


def kernel(**inputs):
    out, _ = _run(inputs, trace=False)
    return out
